# revision 1
# baseline (speedup 1.0000x reference)
"""Trainium2 Bass kernel for 2-layer bidirectional LSTM (B=1024,S=256,F=16,H=64).

Sharding: batch data-parallel across 8 cores (128 batch rows each), weights
replicated. Per core, gate-major layout: gates on partitions, batch on free.

Per direction the 4H=256 gate preactivations are computed as two PSUM tiles
  X = [f;i] (sigmoid), Y = [o;g] (tanh)
via accumulating matmuls (input projection + recurrent projection). The
h-state is stored scaled: h_stored = 2*h_true = (1+tanh(o))*tanh(c), with the
0.5 compensation folded into every consumer weight matrix on the host. This
lets one Sigmoid ACT op and one Tanh ACT op cover all four gates, with the
per-gate bias applied through the ACT bias operand (per-partition vector).

dir f state lives on partitions 0:64, dir r on 64:128, so the layer-0 output
history buffer h0_buf[128, S*B] is directly the layer-1 input, and the two
directions' matmuls occupy disjoint PE row groups (concurrent).
"""
import os
import numpy as np

H = 64
B = 128          # batch per core
S = 256
F = 16
NCORES = 8
FULL_B = 1024
C_OUT = 3

_f32 = None  # set lazily (mybir import)


def _prep_weights(w_ih, w_hh, b_ih, b_hh, scale_in, scale_h):
    """lhsT stacks for X=[f;i], Y=[o;g]; returns dict of host arrays."""
    w_ih = np.asarray(w_ih, np.float32)
    w_hh = np.asarray(w_hh, np.float32)
    b = (np.asarray(b_ih, np.float32) + np.asarray(b_hh, np.float32))
    permX = np.r_[np.arange(64, 128), np.arange(0, 64)]       # [f; i]
    permY = np.r_[np.arange(192, 256), np.arange(128, 192)]   # [o; g]
    out = {}
    # Y stack: o-gate rows pre-scaled by 0.5 so the Tanh ACT yields tanh(o/2),
    # hence 1+tanh(o/2) = 2*sigmoid(o).
    rsX = np.ones((128, 1), np.float32)
    rsY = np.ones((128, 1), np.float32); rsY[0:64] = 0.5
    for name, perm, rs in (("X", permX, rsX), ("Y", permY, rsY)):
        out[f"ih_{name}"] = np.ascontiguousarray((scale_in * rs * w_ih[perm]).T)  # [din,128]
        out[f"hh_{name}"] = np.ascontiguousarray((scale_h * rs * w_hh[perm]).T)   # [64,128]
        out[f"b_{name}"] = np.ascontiguousarray(rs[:, 0] * b[perm])                # [128]
    return out


def _host_prep(inputs):
    """Build all DRAM-side arrays shared by every core (weights) and the
    per-core xT slabs."""
    d = {}
    l0f = _prep_weights(inputs["w_ih_l0"], inputs["w_hh_l0"],
                        inputs["b_ih_l0"], inputs["b_hh_l0"], 1.0, 0.5)
    l0r = _prep_weights(inputs["w_ih_l0r"], inputs["w_hh_l0r"],
                        inputs["b_ih_l0r"], inputs["b_hh_l0r"], 1.0, 0.5)
    l1f = _prep_weights(inputs["w_ih_l1"], inputs["w_hh_l1"],
                        inputs["b_ih_l1"], inputs["b_hh_l1"], 0.5, 0.5)
    l1r = _prep_weights(inputs["w_ih_l1r"], inputs["w_hh_l1r"],
                        inputs["b_ih_l1r"], inputs["b_hh_l1r"], 0.5, 0.5)

    for nm in ("X", "Y"):
        hh0 = np.zeros((128, 128), np.float32)
        hh0[0:64] = l0f[f"hh_{nm}"]
        hh0[64:128] = l0r[f"hh_{nm}"]
        d[f"hh0{nm}"] = hh0
        hh1 = np.zeros((128, 128), np.float32)
        hh1[0:64] = l1f[f"hh_{nm}"]
        hh1[64:128] = l1r[f"hh_{nm}"]
        d[f"hh1{nm}"] = hh1
        ih0 = np.zeros((128, 128), np.float32)
        ih0[0:F] = l0f[f"ih_{nm}"]
        ih0[64:64 + F] = l0r[f"ih_{nm}"]
        d[f"ih0{nm}"] = ih0
        d[f"ih1{nm}f"] = l1f[f"ih_{nm}"]
        d[f"ih1{nm}r"] = l1r[f"ih_{nm}"]

    biases = np.zeros((128, 8), np.float32)
    for li, (lf, lr) in enumerate(((l0f, l0r), (l1f, l1r))):
        for di, wp in enumerate((lf, lr)):
            for si, nm in enumerate(("X", "Y")):
                biases[:, li * 4 + di * 2 + si] = wp[f"b_{nm}"]
    d["biases"] = biases
    d["fcT"] = np.ascontiguousarray(
        (0.5 * np.asarray(inputs["fc_w"], np.float32)).T)           # [128, 3]
    d["fcb"] = np.asarray(inputs["fc_b"], np.float32).reshape(C_OUT, 1)
    return d


def _host_xT(x_core):
    """x_core [B, S, F] -> xT [F, S*B], col = t*B + b."""
    return np.ascontiguousarray(
        np.asarray(x_core, np.float32).transpose(2, 1, 0).reshape(F, -1))


def _patch_tile_drain():
    """This container's walrus rejects instructions carrying multiple sync
    waits ("Too many sync wait commands") — chunk the kernel-tail drain's
    global-clock waits into one drain instruction per semaphore."""
    import concourse.tile as tile
    from concourse.vector_clock import ScopedClock, VectorClock
    if getattr(tile.TileContext, "_drain_patched", False):
        return
    def patched_drain(self, tick_clock, wait_clock):
        gc = tick_clock.global_clock
        n = len(gc)
        procs = [i for i in range(n) if gc[i] > 0]
        chunks = [[p] for p in procs] or [[]]
        for ch in chunks:
            vec = [0] * n
            for p in ch:
                vec[p] = gc[p]
            d = self.nc.sync.drain()
            wait_clock.add_sem_waits(d.ins, ScopedClock({None: VectorClock(vec)}))
        self.nc.all_engine_barrier()
        popped = self.nc._tile_sem_poison_stack.pop()
        assert popped is self._sem_poison
        self.nc.clear_and_free_semaphores(list(self.sems.allocated().values()))
        self.nc.all_engine_barrier()
    tile.TileContext._drain_and_barrier = patched_drain
    tile.TileContext._drain_patched = True


def _split_multi_waits(nc, mybir):
    """This walrus build rejects instructions with more than one sync wait.
    Hoist extra waits onto same-engine NoOp instructions inserted immediately
    before the owning instruction (identical semantics: the engine is
    sequential, so waiting on the prior instruction slot is equivalent)."""
    for f in nc.m.functions:
        for bb in f.blocks:
            out = []
            changed = False
            for inst in bb.instructions:
                si = inst.sync_info
                waits = list(si.on_wait) if si is not None else []
                if len(waits) > 1:
                    changed = True
                    for w in waits[:-1]:
                        nop = mybir.InstNoOp(
                            name=nc.get_next_instruction_name(), ins=[], outs=[])
                        nop.engine = inst.engine
                        nop.sync_info = mybir.SyncInfo(on_wait=[w], on_update=[])
                        out.append(nop)
                    inst.sync_info = mybir.SyncInfo(
                        on_wait=[waits[-1]], on_update=list(si.on_update))
                out.append(inst)
            if changed:
                bb.instructions = out


def build_nc(s_steps=S, use_f32r=False):
    import concourse.bass as bass
    import concourse.tile as tile
    from concourse import mybir
    _patch_tile_drain()

    f32 = mybir.dt.float32
    f32r = mybir.dt.float32r
    AF = mybir.ActivationFunctionType
    ALU = mybir.AluOpType

    def mmcast(ap):
        return ap.bitcast(f32r) if use_f32r else ap

    nc = bass.Bass("TRN2", target_bir_lowering=False, debug=False)

    xT_d = nc.dram_tensor("xT", [F, s_steps * B], f32, kind="ExternalInput")
    wnames = ["hh0X", "hh0Y", "hh1X", "hh1Y", "ih0X", "ih0Y",
              "ih1Xf", "ih1Xr", "ih1Yf", "ih1Yr"]
    wd = {n: nc.dram_tensor(n, [128, 128], f32, kind="ExternalInput")
          for n in wnames}
    bias_d = nc.dram_tensor("biases", [128, 8], f32, kind="ExternalInput")
    fcT_d = nc.dram_tensor("fcT", [128, C_OUT], f32, kind="ExternalInput")
    fcb_d = nc.dram_tensor("fcb", [C_OUT, 1], f32, kind="ExternalInput")
    out_d = nc.dram_tensor("out", [C_OUT, B], f32, kind="ExternalOutput")

    with tile.TileContext(nc) as tc:
        with tc.tile_pool(name="pers", bufs=1) as pers, \
             tc.tile_pool(name="xin", bufs=6) as xin, \
             tc.tile_pool(name="gat", bufs=3) as gat, \
             tc.tile_pool(name="tmp", bufs=3) as tmp, \
             tc.tile_pool(name="ps", bufs=4, space="PSUM") as ps:

            # --- persistent state ---
            h0_buf = pers.tile([128, s_steps * B], f32, tag="h0buf")
            h1_state = pers.tile([128, B], f32, tag="h1s")
            h1_last = pers.tile([128, B], f32, tag="h1l")
            cst = {"f": pers.tile([64, B], f32, tag="cf", name="cf"),
                   "r": pers.tile([64, B], f32, tag="cr", name="cr")}

            # --- weights to SBUF ---
            wsb = {}
            for n in wnames:
                t = pers.tile([128, 128], f32, tag=f"w_{n}", name=f"w_{n}")
                nc.sync.dma_start(out=t[:], in_=wd[n][:])
                wsb[n] = t
            bias_sb = pers.tile([128, 8], f32, tag="bias")
            nc.sync.dma_start(out=bias_sb[:], in_=bias_d[:])
            fcT_sb = pers.tile([128, C_OUT], f32, tag="fcT")
            nc.sync.dma_start(out=fcT_sb[:], in_=fcT_d[:])
            fcb_sb = pers.tile([C_OUT, 1], f32, tag="fcb")
            nc.sync.dma_start(out=fcb_sb[:], in_=fcb_d[:])

            def ts(t):
                return slice(t * B, (t + 1) * B)

            def step(layer, s, dir_, prev_written):
                """One scan step for one direction."""
                di = 0 if dir_ == "f" else 1
                t_proc = s if dir_ == "f" else (s_steps - 1 - s)
                lo, hi = (0, 64) if dir_ == "f" else (64, 128)

                pX = ps.tile([128, B], f32, tag="pX")
                pY = ps.tile([128, B], f32, tag="pY")

                # ---- input projection ----
                if layer == 0:
                    xt = xin.tile([128, B], f32, tag=f"x{dir_}")
                    nc.sync.dma_start(out=xt[lo:lo + F, :],
                                      in_=xT_d[:, ts(t_proc)])
                    rhs_in = xt[lo:lo + F, :]
                    lX, lY = wsb["ih0X"][lo:lo + F, :], wsb["ih0Y"][lo:lo + F, :]
                else:
                    rhs_in = h0_buf[:, ts(t_proc)]
                    sfx = dir_
                    lX, lY = wsb[f"ih1X{sfx}"][:], wsb[f"ih1Y{sfx}"][:]
                first = True
                nc.tensor.matmul(pX[:], mmcast(lX), mmcast(rhs_in),
                                 start=first, stop=(prev_written is None))
                nc.tensor.matmul(pY[:], mmcast(lY), mmcast(rhs_in),
                                 start=first, stop=(prev_written is None))

                # ---- recurrent projection ----
                if prev_written is not None:
                    h_prev = prev_written  # AP [64, B] at partitions lo:hi
                    whX = wsb[f"hh{layer}X"][lo:hi, :]
                    whY = wsb[f"hh{layer}Y"][lo:hi, :]
                    nc.tensor.matmul(pX[:], mmcast(whX), mmcast(h_prev),
                                     start=False, stop=True)
                    nc.tensor.matmul(pY[:], mmcast(whY), mmcast(h_prev),
                                     start=False, stop=True)

                bX = bias_sb[:, layer * 4 + di * 2: layer * 4 + di * 2 + 1]
                bY = bias_sb[:, layer * 4 + di * 2 + 1: layer * 4 + di * 2 + 2]
                sfi = gat.tile([128, B], f32, tag=f"sfi{dir_}")
                tog = gat.tile([128, B], f32, tag=f"tog{dir_}")
                nc.scalar.activation(sfi[:], pX[:], AF.Sigmoid, bias=bX)
                nc.scalar.activation(tog[:], pY[:], AF.Tanh, bias=bY)

                Cd = cst[dir_]
                t1 = tmp.tile([64, B], f32, tag=f"t1{dir_}")
                t2 = tmp.tile([64, B], f32, tag=f"t2{dir_}")
                if prev_written is not None:
                    nc.vector.tensor_mul(t1[:], sfi[0:64, :], Cd[:])
                    nc.vector.tensor_mul(t2[:], sfi[64:128, :], tog[64:128, :])
                    nc.vector.tensor_add(Cd[:], t1[:], t2[:])
                else:
                    nc.vector.tensor_mul(Cd[:], sfi[64:128, :], tog[64:128, :])
                tcv = tmp.tile([64, B], f32, tag=f"tc{dir_}")
                qv = tmp.tile([64, B], f32, tag=f"q{dir_}")
                nc.scalar.activation(tcv[:], Cd[:], AF.Tanh)
                nc.scalar.add(qv[:], tog[0:64, :], 1.0)

                # ---- h write (h_stored = 2h) ----
                if layer == 0:
                    dst = h0_buf[lo:hi, ts(t_proc)]
                    nc.vector.tensor_mul(dst, qv[:], tcv[:])
                    return dst
                else:
                    if dir_ == "f" and s == s_steps - 1:
                        dst = h1_last[0:64, :]
                        nc.vector.tensor_mul(dst, qv[:], tcv[:])
                        return dst
                    dst = h1_state[lo:hi, :]
                    nc.vector.tensor_mul(dst, qv[:], tcv[:])
                    if dir_ == "r" and s == 0:
                        nc.vector.tensor_mul(h1_last[64:128, :], qv[:], tcv[:])
                    return dst

            for layer in (0, 1):
                prev = {"f": None, "r": None}
                for s in range(s_steps):
                    for dir_ in ("f", "r"):
                        prev[dir_] = step(layer, s, dir_, prev[dir_])

            # ---- fc head ----
            pfc = ps.tile([128, B], f32, tag="pX")
            nc.tensor.matmul(pfc[0:C_OUT, :], mmcast(fcT_sb[:]),
                             mmcast(h1_last[:]), start=True, stop=True)
            osb = gat.tile([C_OUT, B], f32, tag="osb")
            nc.scalar.activation(osb[:], pfc[0:C_OUT, :], AF.Identity,
                                 bias=fcb_sb[:, 0:1])
            nc.sync.dma_start(out=out_d[:], in_=osb[:])

    _split_multi_waits(nc, mybir)
    return nc


_cached = {}


def kernel(**inputs):
    from concourse.bass_utils import run_bass_kernel_spmd

    key = "nc"
    if key not in _cached:
        _cached[key] = build_nc(S, use_f32r=False)
    nc = _cached[key]

    shared = _host_prep(inputs)
    x = np.asarray(inputs["x"], np.float32)
    in_maps = []
    for c in range(NCORES):
        m = dict(shared)
        m["xT"] = _host_xT(x[c * B:(c + 1) * B])
        in_maps.append(m)

    res = run_bass_kernel_spmd(nc, in_maps, list(range(NCORES)))
    out = np.concatenate([r["out"].T for r in res.results], axis=0)
    return np.ascontiguousarray(out.astype(np.float32))



# revision 3
# speedup vs baseline: 563.8754x; 563.8754x over previous
"""Trainium2 Bass kernel for 2-layer bidirectional LSTM (B=1024,S=256,F=16,H=64).

Sharding: batch data-parallel across 8 cores (128 batch rows each), weights
replicated. Per core, gate-major layout: gates on partitions, batch on free.

Per direction the 4H=256 gate preactivations are computed as two PSUM tiles
  X = [f;i] (sigmoid), Y = [o;g] (tanh)
via accumulating matmuls (input projection + recurrent projection). The
h-state is stored scaled: h_stored = 2*h_true = (1+tanh(o))*tanh(c), with the
0.5 compensation folded into every consumer weight matrix on the host. This
lets one Sigmoid ACT op and one Tanh ACT op cover all four gates, with the
per-gate bias applied through the ACT bias operand (per-partition vector).

dir f state lives on partitions 0:64, dir r on 64:128, so the layer-0 output
history buffer h0_buf[128, S*B] is directly the layer-1 input, and the two
directions' matmuls occupy disjoint PE row groups (concurrent).
"""
import os
import numpy as np

H = 64
B = 128          # batch per core
S = 256
F = 16
NCORES = 8
FULL_B = 1024
C_OUT = 3

_f32 = None  # set lazily (mybir import)


def _prep_weights(w_ih, w_hh, b_ih, b_hh, scale_in, scale_h):
    """lhsT stacks for X=[f;i], Y=[o;g]; returns dict of host arrays."""
    w_ih = np.asarray(w_ih, np.float32)
    w_hh = np.asarray(w_hh, np.float32)
    b = (np.asarray(b_ih, np.float32) + np.asarray(b_hh, np.float32))
    permX = np.r_[np.arange(64, 128), np.arange(0, 64)]       # [f; i]
    permY = np.r_[np.arange(192, 256), np.arange(128, 192)]   # [o; g]
    out = {}
    # Y stack: o-gate rows pre-scaled by 0.5 so the Tanh ACT yields tanh(o/2),
    # hence 1+tanh(o/2) = 2*sigmoid(o).
    rsX = np.ones((128, 1), np.float32)
    rsY = np.ones((128, 1), np.float32); rsY[0:64] = 0.5
    for name, perm, rs in (("X", permX, rsX), ("Y", permY, rsY)):
        out[f"ih_{name}"] = np.ascontiguousarray((scale_in * rs * w_ih[perm]).T)  # [din,128]
        out[f"hh_{name}"] = np.ascontiguousarray((scale_h * rs * w_hh[perm]).T)   # [64,128]
        out[f"b_{name}"] = np.ascontiguousarray(rs[:, 0] * b[perm])                # [128]
    return out


def _host_prep(inputs):
    """Build all DRAM-side arrays shared by every core (weights) and the
    per-core xT slabs."""
    d = {}
    l0f = _prep_weights(inputs["w_ih_l0"], inputs["w_hh_l0"],
                        inputs["b_ih_l0"], inputs["b_hh_l0"], 1.0, 0.5)
    l0r = _prep_weights(inputs["w_ih_l0r"], inputs["w_hh_l0r"],
                        inputs["b_ih_l0r"], inputs["b_hh_l0r"], 1.0, 0.5)
    l1f = _prep_weights(inputs["w_ih_l1"], inputs["w_hh_l1"],
                        inputs["b_ih_l1"], inputs["b_hh_l1"], 0.5, 0.5)
    l1r = _prep_weights(inputs["w_ih_l1r"], inputs["w_hh_l1r"],
                        inputs["b_ih_l1r"], inputs["b_hh_l1r"], 0.5, 0.5)

    for nm in ("X", "Y"):
        hh0 = np.zeros((128, 128), np.float32)
        hh0[0:64] = l0f[f"hh_{nm}"]
        hh0[64:128] = l0r[f"hh_{nm}"]
        d[f"hh0{nm}"] = hh0
        hh1 = np.zeros((128, 128), np.float32)
        hh1[0:64] = l1f[f"hh_{nm}"]
        hh1[64:128] = l1r[f"hh_{nm}"]
        d[f"hh1{nm}"] = hh1
        ih0 = np.zeros((128, 128), np.float32)
        ih0[0:F] = l0f[f"ih_{nm}"]
        ih0[64:64 + F] = l0r[f"ih_{nm}"]
        d[f"ih0{nm}"] = ih0
        d[f"ih1{nm}f"] = l1f[f"ih_{nm}"]
        d[f"ih1{nm}r"] = l1r[f"ih_{nm}"]

    biases = np.zeros((128, 8), np.float32)
    for li, (lf, lr) in enumerate(((l0f, l0r), (l1f, l1r))):
        for di, wp in enumerate((lf, lr)):
            for si, nm in enumerate(("X", "Y")):
                biases[:, li * 4 + di * 2 + si] = wp[f"b_{nm}"]
    d["biases"] = biases
    d["fcT"] = np.ascontiguousarray(
        (0.5 * np.asarray(inputs["fc_w"], np.float32)).T)           # [128, 3]
    d["fcb"] = np.asarray(inputs["fc_b"], np.float32).reshape(C_OUT, 1)
    return d


def _host_xT(x_core):
    """x_core [B, S, F] -> xT [F, S*B], col = t*B + b."""
    return np.ascontiguousarray(
        np.asarray(x_core, np.float32).transpose(2, 1, 0).reshape(F, -1))


def _patch_tile_drain():
    """This container's walrus rejects instructions carrying multiple sync
    waits ("Too many sync wait commands") — chunk the kernel-tail drain's
    global-clock waits into one drain instruction per semaphore."""
    import concourse.tile as tile
    from concourse.vector_clock import ScopedClock, VectorClock
    if getattr(tile.TileContext, "_drain_patched", False):
        return
    def patched_drain(self, tick_clock, wait_clock):
        gc = tick_clock.global_clock
        n = len(gc)
        procs = [i for i in range(n) if gc[i] > 0]
        chunks = [[p] for p in procs] or [[]]
        for ch in chunks:
            vec = [0] * n
            for p in ch:
                vec[p] = gc[p]
            d = self.nc.sync.drain()
            wait_clock.add_sem_waits(d.ins, ScopedClock({None: VectorClock(vec)}))
        self.nc.all_engine_barrier()
        popped = self.nc._tile_sem_poison_stack.pop()
        assert popped is self._sem_poison
        self.nc.clear_and_free_semaphores(list(self.sems.allocated().values()))
        self.nc.all_engine_barrier()
    tile.TileContext._drain_and_barrier = patched_drain
    tile.TileContext._drain_patched = True


def _split_multi_waits(nc, mybir):
    """This walrus build rejects instructions with more than one sync wait.
    Hoist extra waits onto same-engine NoOp instructions inserted immediately
    before the owning instruction (identical semantics: the engine is
    sequential, so waiting on the prior instruction slot is equivalent)."""
    for f in nc.m.functions:
        for bb in f.blocks:
            out = []
            changed = False
            for inst in bb.instructions:
                si = inst.sync_info
                waits = list(si.on_wait) if si is not None else []
                if len(waits) > 1:
                    changed = True
                    for w in waits[:-1]:
                        nop = mybir.InstNoOp(
                            name=nc.get_next_instruction_name(), ins=[], outs=[])
                        nop.engine = inst.engine
                        nop.sync_info = mybir.SyncInfo(on_wait=[w], on_update=[])
                        out.append(nop)
                    inst.sync_info = mybir.SyncInfo(
                        on_wait=[waits[-1]], on_update=list(si.on_update))
                out.append(inst)
            if changed:
                bb.instructions = out


def build_nc(s_steps=S, use_f32r=False):
    import concourse.bass as bass
    import concourse.tile as tile
    from concourse import mybir
    _patch_tile_drain()

    f32 = mybir.dt.float32
    f32r = mybir.dt.float32r
    AF = mybir.ActivationFunctionType
    ALU = mybir.AluOpType

    def mmcast(ap):
        return ap.bitcast(f32r) if use_f32r else ap

    nc = bass.Bass("TRN2", target_bir_lowering=False, debug=False)

    xT_d = nc.dram_tensor("xT", [F, s_steps * B], f32, kind="ExternalInput")
    wnames = ["hh0X", "hh0Y", "hh1X", "hh1Y", "ih0X", "ih0Y",
              "ih1Xf", "ih1Xr", "ih1Yf", "ih1Yr"]
    wd = {n: nc.dram_tensor(n, [128, 128], f32, kind="ExternalInput")
          for n in wnames}
    bias_d = nc.dram_tensor("biases", [128, 8], f32, kind="ExternalInput")
    fcT_d = nc.dram_tensor("fcT", [128, C_OUT], f32, kind="ExternalInput")
    fcb_d = nc.dram_tensor("fcb", [C_OUT, 1], f32, kind="ExternalInput")
    out_d = nc.dram_tensor("out", [C_OUT, B], f32, kind="ExternalOutput")

    with tile.TileContext(nc) as tc:
        with tc.tile_pool(name="pers", bufs=1) as pers, \
             tc.tile_pool(name="xin", bufs=6) as xin, \
             tc.tile_pool(name="gat", bufs=3) as gat, \
             tc.tile_pool(name="tmp", bufs=3) as tmp, \
             tc.tile_pool(name="ps", bufs=4, space="PSUM") as ps:

            # --- persistent state ---
            h0_buf = pers.tile([128, s_steps * B], f32, tag="h0buf")
            h1_state = pers.tile([128, B], f32, tag="h1s")
            h1_last = pers.tile([128, B], f32, tag="h1l")
            cst = {"f": pers.tile([64, B], f32, tag="cf", name="cf"),
                   "r": pers.tile([64, B], f32, tag="cr", name="cr")}

            # --- weights to SBUF ---
            wsb = {}
            for n in wnames:
                t = pers.tile([128, 128], f32, tag=f"w_{n}", name=f"w_{n}")
                nc.sync.dma_start(out=t[:], in_=wd[n][:])
                wsb[n] = t
            bias_sb = pers.tile([128, 8], f32, tag="bias")
            nc.sync.dma_start(out=bias_sb[:], in_=bias_d[:])
            fcT_sb = pers.tile([128, C_OUT], f32, tag="fcT")
            nc.sync.dma_start(out=fcT_sb[:], in_=fcT_d[:])
            fcb_sb = pers.tile([C_OUT, 1], f32, tag="fcb")
            nc.sync.dma_start(out=fcb_sb[:], in_=fcb_d[:])

            def ts(t):
                return slice(t * B, (t + 1) * B)

            def step(layer, s, dir_, prev_written):
                """One scan step for one direction."""
                di = 0 if dir_ == "f" else 1
                t_proc = s if dir_ == "f" else (s_steps - 1 - s)
                lo, hi = (0, 64) if dir_ == "f" else (64, 128)

                pX = ps.tile([128, B], f32, tag="pX")
                pY = ps.tile([128, B], f32, tag="pY")

                # ---- input projection ----
                if layer == 0:
                    xt = xin.tile([128, B], f32, tag=f"x{dir_}")
                    nc.sync.dma_start(out=xt[lo:lo + F, :],
                                      in_=xT_d[:, ts(t_proc)])
                    rhs_in = xt[lo:lo + F, :]
                    lX, lY = wsb["ih0X"][lo:lo + F, :], wsb["ih0Y"][lo:lo + F, :]
                else:
                    rhs_in = h0_buf[:, ts(t_proc)]
                    sfx = dir_
                    lX, lY = wsb[f"ih1X{sfx}"][:], wsb[f"ih1Y{sfx}"][:]
                first = True
                nc.tensor.matmul(pX[:], mmcast(lX), mmcast(rhs_in),
                                 start=first, stop=(prev_written is None))
                nc.tensor.matmul(pY[:], mmcast(lY), mmcast(rhs_in),
                                 start=first, stop=(prev_written is None))

                # ---- recurrent projection ----
                if prev_written is not None:
                    h_prev = prev_written  # AP [64, B] at partitions lo:hi
                    whX = wsb[f"hh{layer}X"][lo:hi, :]
                    whY = wsb[f"hh{layer}Y"][lo:hi, :]
                    nc.tensor.matmul(pX[:], mmcast(whX), mmcast(h_prev),
                                     start=False, stop=True)
                    nc.tensor.matmul(pY[:], mmcast(whY), mmcast(h_prev),
                                     start=False, stop=True)

                bX = bias_sb[:, layer * 4 + di * 2: layer * 4 + di * 2 + 1]
                bY = bias_sb[:, layer * 4 + di * 2 + 1: layer * 4 + di * 2 + 2]
                sfi = gat.tile([128, B], f32, tag=f"sfi{dir_}")
                tog = gat.tile([128, B], f32, tag=f"tog{dir_}")
                nc.scalar.activation(sfi[:], pX[:], AF.Sigmoid, bias=bX)
                nc.scalar.activation(tog[:], pY[:], AF.Tanh, bias=bY)

                Cd = cst[dir_]
                t1 = tmp.tile([64, B], f32, tag=f"t1{dir_}")
                t2 = tmp.tile([64, B], f32, tag=f"t2{dir_}")
                if prev_written is not None:
                    nc.vector.tensor_mul(t1[:], sfi[0:64, :], Cd[:])
                    nc.vector.tensor_mul(t2[:], sfi[64:128, :], tog[64:128, :])
                    nc.vector.tensor_add(Cd[:], t1[:], t2[:])
                else:
                    nc.vector.tensor_mul(Cd[:], sfi[64:128, :], tog[64:128, :])
                tcv = tmp.tile([64, B], f32, tag=f"tc{dir_}")
                qv = tmp.tile([64, B], f32, tag=f"q{dir_}")
                nc.scalar.activation(tcv[:], Cd[:], AF.Tanh)
                nc.scalar.add(qv[:], tog[0:64, :], 1.0)

                # ---- h write (h_stored = 2h) ----
                if layer == 0:
                    dst = h0_buf[lo:hi, ts(t_proc)]
                    nc.vector.tensor_mul(dst, qv[:], tcv[:])
                    return dst
                else:
                    if dir_ == "f" and s == s_steps - 1:
                        dst = h1_last[0:64, :]
                        nc.vector.tensor_mul(dst, qv[:], tcv[:])
                        return dst
                    dst = h1_state[lo:hi, :]
                    nc.vector.tensor_mul(dst, qv[:], tcv[:])
                    if dir_ == "r" and s == 0:
                        nc.vector.tensor_mul(h1_last[64:128, :], qv[:], tcv[:])
                    return dst

            for layer in (0, 1):
                prev = {"f": None, "r": None}
                for s in range(s_steps):
                    for dir_ in ("f", "r"):
                        prev[dir_] = step(layer, s, dir_, prev[dir_])

            # ---- fc head ----
            pfc = ps.tile([128, B], f32, tag="pX")
            nc.tensor.matmul(pfc[0:C_OUT, :], mmcast(fcT_sb[:]),
                             mmcast(h1_last[:]), start=True, stop=True)
            osb = gat.tile([C_OUT, B], f32, tag="osb")
            nc.scalar.activation(osb[:], pfc[0:C_OUT, :], AF.Identity,
                                 bias=fcb_sb[:, 0:1])
            nc.sync.dma_start(out=out_d[:], in_=osb[:])

    _split_multi_waits(nc, mybir)
    return nc


_cached = {}


def _get_compiled():
    """Build the Bass module once and compile it into a reusable jitted
    executable over the 8-core mesh (run_bass_via_pjrt re-traces and
    re-lowers on every call; this path does it once per process)."""
    if "exe" in _cached:
        return _cached["exe"]

    import jax
    from jax.sharding import Mesh, NamedSharding, PartitionSpec
    from jax.experimental.shard_map import shard_map as _shard_map
    from concourse import mybir
    from concourse.bass2jax import (_bass_exec_p, install_neuronx_cc_hook,
                                    partition_id_tensor)

    nc = build_nc(S, use_f32r=False)
    install_neuronx_cc_hook()

    partition_name = (nc.partition_id_tensor.name
                      if nc.partition_id_tensor else None)
    in_names, out_names, out_avals, zero_outs = [], [], [], []
    for alloc in nc.m.functions[0].allocations:
        if not isinstance(alloc, mybir.MemoryLocationSet):
            continue
        name = alloc.memorylocations[0].name
        if alloc.kind == "ExternalInput":
            if name != partition_name:
                in_names.append(name)
        elif alloc.kind == "ExternalOutput":
            shape = tuple(alloc.tensor_shape)
            dtype = mybir.dt.np(alloc.dtype)
            out_names.append(name)
            out_avals.append(jax.core.ShapedArray(shape, dtype))
            zero_outs.append(np.zeros(shape, dtype))
    n_params = len(in_names)
    in_names_all = list(in_names) + out_names
    if partition_name is not None:
        in_names_all.append(partition_name)

    def _body(*args):
        operands = list(args)
        if partition_name is not None:
            operands.append(partition_id_tensor())
        return tuple(_bass_exec_p.bind(
            *operands,
            out_avals=tuple(out_avals),
            in_names=tuple(in_names_all),
            out_names=tuple(out_names),
            lowering_input_output_aliases=(),
            sim_require_finite=True,
            sim_require_nnan=True,
            nc=nc,
        ))

    devices = jax.devices()[:NCORES]
    mesh = Mesh(np.asarray(devices), ("core",))
    spec = PartitionSpec("core")
    sharded = jax.jit(
        _shard_map(_body, mesh=mesh,
                   in_specs=(spec,) * (n_params + len(out_avals)),
                   out_specs=(spec,) * len(out_names),
                   check_rep=False),
        keep_unused=True,
    )
    exe = {
        "nc": nc,
        "fn": sharded,
        "in_names": in_names,
        "out_names": out_names,
        "zero_outs": zero_outs,
        "sharding": NamedSharding(mesh, spec),
    }
    _cached["exe"] = exe
    _cached["nc"] = nc
    return exe


def make_in_maps(inputs):
    """Full inputs -> per-core DRAM tensor maps (host-side prep)."""
    shared = _host_prep(inputs)
    x = np.asarray(inputs["x"], np.float32)
    in_maps = []
    for c in range(NCORES):
        m = dict(shared)
        m["xT"] = _host_xT(x[c * B:(c + 1) * B])
        in_maps.append(m)
    return in_maps


def concat_inputs(exe, in_maps):
    """Per-core maps -> the flat concatenated operand list fed to exe['fn']."""
    per_core = [[np.asarray(m[n]) for n in exe["in_names"]] for m in in_maps]
    ops = [np.concatenate([per_core[c][i] for c in range(NCORES)], axis=0)
           for i in range(len(exe["in_names"]))]
    ops += [np.concatenate([z] * NCORES, axis=0) for z in exe["zero_outs"]]
    return ops


def run_ops(exe, ops):
    """Execute; returns the full [FULL_B, C_OUT] output."""
    import jax
    outs = exe["fn"](*ops)
    jax.block_until_ready(outs)
    # out tensor is [C_OUT, B] per core, concatenated on axis 0 over cores.
    o = np.asarray(outs[exe["out_names"].index("out")])
    o = o.reshape(NCORES, C_OUT, B)
    return np.ascontiguousarray(
        np.concatenate([o[c].T for c in range(NCORES)], axis=0)
        .astype(np.float32))


def kernel(**inputs):
    exe = _get_compiled()
    return run_ops(exe, concat_inputs(exe, make_in_maps(inputs)))



# revision 9
# speedup vs baseline: 772.1791x; 1.3694x over previous
"""Trainium2 Bass kernel for 2-layer bidirectional LSTM (B=1024,S=256,F=16,H=64).

Sharding: batch data-parallel across 8 cores (128 batch rows each), weights
replicated. Per core, gate-major layout: gates on partitions, batch on free.

Per direction the 4H=256 gate preactivations are computed as two PSUM tiles
  X = [f;i] (sigmoid), Y = [o;g] (tanh)
via accumulating matmuls (input projection + recurrent projection). The
h-state is stored scaled: h_stored = 2*h_true = (1+tanh(o))*tanh(c), with the
0.5 compensation folded into every consumer weight matrix on the host. This
lets one Sigmoid ACT op and one Tanh ACT op cover all four gates, with the
per-gate bias applied through the ACT bias operand (per-partition vector).

dir f state lives on partitions 0:64, dir r on 64:128, so the layer-0 output
history buffer h0_buf[128, S*B] is directly the layer-1 input, and the two
directions' matmuls occupy disjoint PE row groups (concurrent).
"""
import os
import numpy as np

H = 64
B = 128          # batch per core
S = 256
F = 16
NCORES = 8
FULL_B = 1024
C_OUT = 3

_f32 = None  # set lazily (mybir import)


def _prep_weights(w_ih, w_hh, b_ih, b_hh, scale_in, scale_h):
    """lhsT stacks for X=[f;i], Y=[o;g]; returns dict of host arrays."""
    w_ih = np.asarray(w_ih, np.float32)
    w_hh = np.asarray(w_hh, np.float32)
    b = (np.asarray(b_ih, np.float32) + np.asarray(b_hh, np.float32))
    permX = np.r_[np.arange(64, 128), np.arange(0, 64)]       # [f; i]
    permY = np.r_[np.arange(192, 256), np.arange(128, 192)]   # [o; g]
    out = {}
    # Y stack: o-gate rows pre-scaled by 0.5 so the Tanh ACT yields tanh(o/2),
    # hence 1+tanh(o/2) = 2*sigmoid(o).
    rsX = np.ones((128, 1), np.float32)
    rsY = np.ones((128, 1), np.float32); rsY[0:64] = 0.5
    for name, perm, rs in (("X", permX, rsX), ("Y", permY, rsY)):
        out[f"ih_{name}"] = np.ascontiguousarray((scale_in * rs * w_ih[perm]).T)  # [din,128]
        out[f"hh_{name}"] = np.ascontiguousarray((scale_h * rs * w_hh[perm]).T)   # [64,128]
        out[f"b_{name}"] = np.ascontiguousarray(rs[:, 0] * b[perm])                # [128]
    return out


def _host_prep(inputs):
    """Build all DRAM-side arrays shared by every core (weights) and the
    per-core xT slabs."""
    d = {}
    l0f = _prep_weights(inputs["w_ih_l0"], inputs["w_hh_l0"],
                        inputs["b_ih_l0"], inputs["b_hh_l0"], 1.0, 0.5)
    l0r = _prep_weights(inputs["w_ih_l0r"], inputs["w_hh_l0r"],
                        inputs["b_ih_l0r"], inputs["b_hh_l0r"], 1.0, 0.5)
    l1f = _prep_weights(inputs["w_ih_l1"], inputs["w_hh_l1"],
                        inputs["b_ih_l1"], inputs["b_hh_l1"], 0.5, 0.5)
    l1r = _prep_weights(inputs["w_ih_l1r"], inputs["w_hh_l1r"],
                        inputs["b_ih_l1r"], inputs["b_hh_l1r"], 0.5, 0.5)

    for nm in ("X", "Y"):
        hh0 = np.zeros((128, 128), np.float32)
        hh0[0:64] = l0f[f"hh_{nm}"]
        hh0[64:128] = l0r[f"hh_{nm}"]
        d[f"hh0{nm}"] = hh0
        hh1 = np.zeros((128, 128), np.float32)
        hh1[0:64] = l1f[f"hh_{nm}"]
        hh1[64:128] = l1r[f"hh_{nm}"]
        d[f"hh1{nm}"] = hh1
        ih0 = np.zeros((128, 128), np.float32)
        ih0[0:F] = l0f[f"ih_{nm}"]
        ih0[64:64 + F] = l0r[f"ih_{nm}"]
        d[f"ih0{nm}"] = ih0
        d[f"ih1{nm}f"] = l1f[f"ih_{nm}"]
        d[f"ih1{nm}r"] = l1r[f"ih_{nm}"]

    biases = np.zeros((128, 8), np.float32)
    for li, (lf, lr) in enumerate(((l0f, l0r), (l1f, l1r))):
        for di, wp in enumerate((lf, lr)):
            for si, nm in enumerate(("X", "Y")):
                biases[:, li * 4 + di * 2 + si] = wp[f"b_{nm}"]
    d["biases"] = biases
    d["fcT"] = np.ascontiguousarray(
        (0.5 * np.asarray(inputs["fc_w"], np.float32)).T)           # [128, 3]
    d["fcb"] = np.asarray(inputs["fc_b"], np.float32).reshape(C_OUT, 1)
    return d


def _host_xT(x_core):
    """x_core [B, S, F] -> xT [F, S*B], col = t*B + b."""
    return np.ascontiguousarray(
        np.asarray(x_core, np.float32).transpose(2, 1, 0).reshape(F, -1))


def _patch_tile_drain():
    """This container's walrus rejects instructions carrying multiple sync
    waits ("Too many sync wait commands") — chunk the kernel-tail drain's
    global-clock waits into one drain instruction per semaphore."""
    import concourse.tile as tile
    from concourse.vector_clock import ScopedClock, VectorClock
    if getattr(tile.TileContext, "_drain_patched", False):
        return
    def patched_drain(self, tick_clock, wait_clock):
        gc = tick_clock.global_clock
        n = len(gc)
        procs = [i for i in range(n) if gc[i] > 0]
        chunks = [[p] for p in procs] or [[]]
        for ch in chunks:
            vec = [0] * n
            for p in ch:
                vec[p] = gc[p]
            d = self.nc.sync.drain()
            wait_clock.add_sem_waits(d.ins, ScopedClock({None: VectorClock(vec)}))
        self.nc.all_engine_barrier()
        popped = self.nc._tile_sem_poison_stack.pop()
        assert popped is self._sem_poison
        self.nc.clear_and_free_semaphores(list(self.sems.allocated().values()))
        self.nc.all_engine_barrier()
    tile.TileContext._drain_and_barrier = patched_drain
    tile.TileContext._drain_patched = True


def _split_multi_waits(nc, mybir):
    """This walrus build rejects instructions with more than one sync wait.
    Hoist extra waits onto same-engine NoOp instructions inserted immediately
    before the owning instruction (identical semantics: the engine is
    sequential, so waiting on the prior instruction slot is equivalent)."""
    for f in nc.m.functions:
        for bb in f.blocks:
            out = []
            changed = False
            for inst in bb.instructions:
                si = inst.sync_info
                waits = list(si.on_wait) if si is not None else []
                if len(waits) > 1:
                    changed = True
                    for w in waits[:-1]:
                        nop = mybir.InstNoOp(
                            name=nc.get_next_instruction_name(), ins=[], outs=[])
                        nop.engine = inst.engine
                        nop.sync_info = mybir.SyncInfo(on_wait=[w], on_update=[])
                        out.append(nop)
                    inst.sync_info = mybir.SyncInfo(
                        on_wait=[waits[-1]], on_update=list(si.on_update))
                out.append(inst)
            if changed:
                bb.instructions = out


def build_nc(s_steps=S, use_f32r=False):
    import concourse.bass as bass
    import concourse.tile as tile
    from concourse import mybir
    _patch_tile_drain()

    f32 = mybir.dt.float32
    f32r = mybir.dt.float32r
    AF = mybir.ActivationFunctionType
    ALU = mybir.AluOpType

    def mmcast(ap):
        return ap.bitcast(f32r) if use_f32r else ap

    nc = bass.Bass("TRN2", target_bir_lowering=False, debug=False)

    xT_d = nc.dram_tensor("xT", [F, s_steps * B], f32, kind="ExternalInput")
    wnames = ["hh0X", "hh0Y", "hh1X", "hh1Y", "ih0X", "ih0Y",
              "ih1Xf", "ih1Xr", "ih1Yf", "ih1Yr"]
    wd = {n: nc.dram_tensor(n, [128, 128], f32, kind="ExternalInput")
          for n in wnames}
    bias_d = nc.dram_tensor("biases", [128, 8], f32, kind="ExternalInput")
    fcT_d = nc.dram_tensor("fcT", [128, C_OUT], f32, kind="ExternalInput")
    fcb_d = nc.dram_tensor("fcb", [C_OUT, 1], f32, kind="ExternalInput")
    out_d = nc.dram_tensor("out", [C_OUT, B], f32, kind="ExternalOutput")

    with tile.TileContext(nc) as tc:
        with tc.tile_pool(name="pers", bufs=1) as pers, \
             tc.tile_pool(name="xin", bufs=6) as xin, \
             tc.tile_pool(name="gat", bufs=3) as gat, \
             tc.tile_pool(name="tmp", bufs=3) as tmp, \
             tc.tile_pool(name="ps", bufs=4, space="PSUM") as ps:

            # --- persistent state ---
            h0_buf = pers.tile([128, s_steps * B], f32, tag="h0buf")
            h1_state = pers.tile([128, B], f32, tag="h1s")
            h1_last = pers.tile([128, B], f32, tag="h1l")
            cst = {"f": pers.tile([64, B], f32, tag="cf", name="cf"),
                   "r": pers.tile([64, B], f32, tag="cr", name="cr")}

            # --- weights to SBUF ---
            wsb = {}
            for n in wnames:
                t = pers.tile([128, 128], f32, tag=f"w_{n}", name=f"w_{n}")
                nc.sync.dma_start(out=t[:], in_=wd[n][:])
                wsb[n] = t
            bias_sb = pers.tile([128, 8], f32, tag="bias")
            nc.sync.dma_start(out=bias_sb[:], in_=bias_d[:])
            fcT_sb = pers.tile([128, C_OUT], f32, tag="fcT")
            nc.sync.dma_start(out=fcT_sb[:], in_=fcT_d[:])
            fcb_sb = pers.tile([C_OUT, 1], f32, tag="fcb")
            nc.sync.dma_start(out=fcb_sb[:], in_=fcb_d[:])

            def ts(t):
                return slice(t * B, (t + 1) * B)

            def step(layer, s, dir_, prev_written):
                """One scan step for one direction."""
                di = 0 if dir_ == "f" else 1
                t_proc = s if dir_ == "f" else (s_steps - 1 - s)
                lo, hi = (0, 64) if dir_ == "f" else (64, 128)

                pX = ps.tile([128, B], f32, tag="pX")
                pY = ps.tile([128, B], f32, tag="pY")

                # ---- input projection ----
                if layer == 0:
                    xt = xin.tile([128, B], f32, tag=f"x{dir_}")
                    nc.sync.dma_start(out=xt[lo:lo + F, :],
                                      in_=xT_d[:, ts(t_proc)])
                    rhs_in = xt[lo:lo + F, :]
                    lX, lY = wsb["ih0X"][lo:lo + F, :], wsb["ih0Y"][lo:lo + F, :]
                else:
                    rhs_in = h0_buf[:, ts(t_proc)]
                    sfx = dir_
                    lX, lY = wsb[f"ih1X{sfx}"][:], wsb[f"ih1Y{sfx}"][:]
                first = True
                nc.tensor.matmul(pX[:], mmcast(lX), mmcast(rhs_in),
                                 start=first, stop=(prev_written is None))
                nc.tensor.matmul(pY[:], mmcast(lY), mmcast(rhs_in),
                                 start=first, stop=(prev_written is None))

                # ---- recurrent projection ----
                if prev_written is not None:
                    h_prev = prev_written  # AP [64, B] at partitions lo:hi
                    whX = wsb[f"hh{layer}X"][lo:hi, :]
                    whY = wsb[f"hh{layer}Y"][lo:hi, :]
                    nc.tensor.matmul(pX[:], mmcast(whX), mmcast(h_prev),
                                     start=False, stop=True)
                    nc.tensor.matmul(pY[:], mmcast(whY), mmcast(h_prev),
                                     start=False, stop=True)

                bX = bias_sb[:, layer * 4 + di * 2: layer * 4 + di * 2 + 1]
                bY = bias_sb[:, layer * 4 + di * 2 + 1: layer * 4 + di * 2 + 2]
                sfi = gat.tile([128, B], f32, tag=f"sfi{dir_}")
                tog = gat.tile([128, B], f32, tag=f"tog{dir_}")
                nc.scalar.activation(sfi[:], pX[:], AF.Sigmoid, bias=bX)
                nc.scalar.activation(tog[:], pY[:], AF.Tanh, bias=bY)

                Cd = cst[dir_]
                t1 = tmp.tile([64, B], f32, tag=f"t1{dir_}")
                t2 = tmp.tile([64, B], f32, tag=f"t2{dir_}")
                if prev_written is not None:
                    nc.vector.tensor_mul(t1[:], sfi[0:64, :], Cd[:])
                    nc.vector.tensor_mul(t2[:], sfi[64:128, :], tog[64:128, :])
                    nc.vector.tensor_add(Cd[:], t1[:], t2[:])
                else:
                    nc.vector.tensor_mul(Cd[:], sfi[64:128, :], tog[64:128, :])
                tcv = tmp.tile([64, B], f32, tag=f"tc{dir_}")
                qv = tmp.tile([64, B], f32, tag=f"q{dir_}")
                nc.scalar.activation(tcv[:], Cd[:], AF.Tanh)
                nc.scalar.add(qv[:], tog[0:64, :], 1.0)

                # ---- h write (h_stored = 2h) ----
                if layer == 0:
                    dst = h0_buf[lo:hi, ts(t_proc)]
                    nc.vector.tensor_mul(dst, qv[:], tcv[:])
                    return dst
                else:
                    if dir_ == "f" and s == s_steps - 1:
                        dst = h1_last[0:64, :]
                        nc.vector.tensor_mul(dst, qv[:], tcv[:])
                        return dst
                    dst = h1_state[lo:hi, :]
                    nc.vector.tensor_mul(dst, qv[:], tcv[:])
                    if dir_ == "r" and s == 0:
                        nc.vector.tensor_mul(h1_last[64:128, :], qv[:], tcv[:])
                    return dst

            for layer in (0, 1):
                prev = {"f": None, "r": None}
                for s in range(s_steps):
                    for dir_ in ("f", "r"):
                        prev[dir_] = step(layer, s, dir_, prev[dir_])

            # ---- fc head ----
            pfc = ps.tile([128, B], f32, tag="pX")
            nc.tensor.matmul(pfc[0:C_OUT, :], mmcast(fcT_sb[:]),
                             mmcast(h1_last[:]), start=True, stop=True)
            osb = gat.tile([C_OUT, B], f32, tag="osb")
            nc.scalar.activation(osb[:], pfc[0:C_OUT, :], AF.Identity,
                                 bias=fcb_sb[:, 0:1])
            nc.sync.dma_start(out=out_d[:], in_=osb[:])

    _split_multi_waits(nc, mybir)
    return nc


G4 = 4  # timesteps per PSUM group


def _prep_dir(w_ih, w_hh, b_ih, b_hh, scale_in, scale_h):
    """Per-direction weight prep (same math as _prep_weights, but returning
    the ih and hh lhsT stacks separately)."""
    w_ih = np.asarray(w_ih, np.float32)
    w_hh = np.asarray(w_hh, np.float32)
    b = np.asarray(b_ih, np.float32) + np.asarray(b_hh, np.float32)
    permX = np.r_[np.arange(64, 128), np.arange(0, 64)]       # [f; i]
    permY = np.r_[np.arange(192, 256), np.arange(128, 192)]   # [o; g]
    rsX = np.ones((128, 1), np.float32)
    rsY = np.ones((128, 1), np.float32); rsY[0:64] = 0.5
    out = {}
    for nm, perm, rs in (("X", permX, rsX), ("Y", permY, rsY)):
        out[f"ih_{nm}"] = np.ascontiguousarray((scale_in * rs * w_ih[perm]).T)
        out[f"hh_{nm}"] = np.ascontiguousarray((scale_h * rs * w_hh[perm]).T)
        out[f"b_{nm}"] = np.ascontiguousarray(rs[:, 0] * b[perm])
    return out


def _host_prep2(inputs, mm_bf16=True):
    """DRAM arrays for the v2 kernel. hh lhsT live in [128,128] containers
    (dir f rows 0:64, dir r rows 64:128) so lhsT partitions align with the
    rhs h slices, as in v1."""
    import ml_dtypes
    mmdt = ml_dtypes.bfloat16 if mm_bf16 else np.float32
    l0 = {"f": _prep_dir(inputs["w_ih_l0"], inputs["w_hh_l0"],
                         inputs["b_ih_l0"], inputs["b_hh_l0"], 1.0, 0.5),
          "r": _prep_dir(inputs["w_ih_l0r"], inputs["w_hh_l0r"],
                         inputs["b_ih_l0r"], inputs["b_hh_l0r"], 1.0, 0.5)}
    l1 = {"f": _prep_dir(inputs["w_ih_l1"], inputs["w_hh_l1"],
                         inputs["b_ih_l1"], inputs["b_hh_l1"], 0.5, 0.5),
          "r": _prep_dir(inputs["w_ih_l1r"], inputs["w_hh_l1r"],
                         inputs["b_ih_l1r"], inputs["b_hh_l1r"], 0.5, 0.5)}
    d = {}
    for nm in ("X", "Y"):
        for dir_ in ("f", "r"):
            d[f"ih0{nm}{dir_}"] = l0[dir_][f"ih_{nm}"].astype(np.float32)
            d[f"ih1{nm}{dir_}"] = l1[dir_][f"ih_{nm}"].astype(mmdt)
        for li, lw in ((0, l0), (1, l1)):
            hh = np.zeros((128, 128), np.float32)
            hh[0:64] = lw["f"][f"hh_{nm}"]
            hh[64:128] = lw["r"][f"hh_{nm}"]
            d[f"hh{li}{nm}"] = hh.astype(mmdt)
    biases = np.zeros((128, 8), np.float32)
    for li, lw in ((0, l0), (1, l1)):
        for di, dir_ in enumerate(("f", "r")):
            for si, nm in enumerate(("X", "Y")):
                biases[:, li * 4 + di * 2 + si] = lw[dir_][f"b_{nm}"]
    d["biases"] = biases
    d["fcT"] = np.ascontiguousarray(
        (0.5 * np.asarray(inputs["fc_w"], np.float32)).T)
    d["fcb"] = np.asarray(inputs["fc_b"], np.float32).reshape(C_OUT, 1)
    return d


def build_nc2(s_steps=S, mm_bf16=True, ew_bf16=True, c_bf16=False):
    """v2: group-of-4 input-projection GEMMs (free=4B=512 so f32r runs at
    1 cyc/row), bf16 recurrent matmuls + h storage, 3 ACT ops per
    direction-step, fused (tog+1)*tanh(c) via scalar_tensor_tensor on DVE,
    i*g product on GPSIMD. The reverse chain writes its PSUM group columns
    in reversed order (m = G4-1-j) so its input-projection GEMM reads a
    forward-contiguous block."""
    import concourse.bass as bass
    import concourse.tile as tile
    from concourse import mybir
    _patch_tile_drain()

    f32 = mybir.dt.float32
    f32r = mybir.dt.float32r
    bf16 = mybir.dt.bfloat16
    AF = mybir.ActivationFunctionType
    ALU = mybir.AluOpType
    MMB = bf16 if mm_bf16 else f32
    EWD = bf16 if ew_bf16 else f32
    CD = bf16 if c_bf16 else f32
    NG = s_steps // G4

    def mm(ap):
        return ap.bitcast(f32r) if ap.dtype == f32 else ap

    nc = bass.Bass("TRN2", target_bir_lowering=False, debug=False)

    xT_d = nc.dram_tensor("xT", [F, s_steps * B], f32r, kind="ExternalInput")
    wd = {}
    for nm in ("X", "Y"):
        for dir_ in ("f", "r"):
            wd[f"ih0{nm}{dir_}"] = nc.dram_tensor(
                f"ih0{nm}{dir_}", [F, 128], f32r, kind="ExternalInput")
            wd[f"ih1{nm}{dir_}"] = nc.dram_tensor(
                f"ih1{nm}{dir_}", [128, 128], MMB, kind="ExternalInput")
        for li in (0, 1):
            wd[f"hh{li}{nm}"] = nc.dram_tensor(
                f"hh{li}{nm}", [128, 128], MMB, kind="ExternalInput")
    bias_d = nc.dram_tensor("biases", [128, 8], f32, kind="ExternalInput")
    fcT_d = nc.dram_tensor("fcT", [128, C_OUT], f32, kind="ExternalInput")
    fcb_d = nc.dram_tensor("fcb", [C_OUT, 1], f32, kind="ExternalInput")
    out_d = nc.dram_tensor("out", [C_OUT, B], f32, kind="ExternalOutput")

    with tile.TileContext(nc) as tc:
        with tc.tile_pool(name="pers", bufs=1) as pers, \
             tc.tile_pool(name="xin", bufs=3) as xin, \
             tc.tile_pool(name="gat", bufs=4) as gat, \
             tc.tile_pool(name="tmp", bufs=4) as tmp, \
             tc.tile_pool(name="ps", bufs=2, space="PSUM") as ps:

            h0_buf = pers.tile([128, s_steps * B], MMB, tag="h0buf")
            h1state = pers.tile([128, B], MMB, tag="h1s")
            h1last = pers.tile([128, B], f32, tag="h1l")
            cst = {"f": pers.tile([64, B], CD, tag="cf", name="cf"),
                   "r": pers.tile([64, B], CD, tag="cr", name="cr")}

            wsb = {}
            for n, t_ in wd.items():
                w = pers.tile(list(t_.shape), t_.dtype, tag=f"w_{n}",
                              name=f"w_{n}")
                nc.sync.dma_start(out=w[:], in_=t_[:])
                wsb[n] = w
            bias_sb = pers.tile([128, 8], f32, tag="bias")
            nc.sync.dma_start(out=bias_sb[:], in_=bias_d[:])
            fcT_sb = pers.tile([128, C_OUT], f32, tag="fcT")
            nc.sync.dma_start(out=fcT_sb[:], in_=fcT_d[:])
            fcb_sb = pers.tile([C_OUT, 1], f32, tag="fcb")
            nc.sync.dma_start(out=fcb_sb[:], in_=fcb_d[:])

            for layer in (0, 1):
                for g in range(NG):
                    s0 = g * G4
                    # ---- group input-projection GEMMs ----
                    pt = {}
                    for nm in ("X", "Y"):
                        for dir_ in ("f", "r"):
                            pt[nm + dir_] = ps.tile([128, G4 * B], f32,
                                                    tag=f"p{nm}{dir_}",
                                                    name=f"p{nm}{dir_}")
                    for dir_ in ("f", "r"):
                        if dir_ == "f":
                            cols = slice(s0 * B, (s0 + G4) * B)
                        else:
                            cols = slice((s_steps - G4 - s0) * B,
                                         (s_steps - s0) * B)
                        if layer == 0:
                            xg = xin.tile([F, G4 * B], f32r, tag=f"x{dir_}",
                                          name=f"x{dir_}")
                            nc.sync.dma_start(out=xg[:], in_=xT_d[:, cols])
                            rhs = xg[:]
                            lX = wsb[f"ih0X{dir_}"][:]
                            lY = wsb[f"ih0Y{dir_}"][:]
                        else:
                            rhs = h0_buf[:, cols]
                            lX = wsb[f"ih1X{dir_}"][:]
                            lY = wsb[f"ih1Y{dir_}"][:]
                        nc.tensor.matmul(pt["X" + dir_][:], mm(lX), mm(rhs),
                                         start=True, stop=True,
                                         skip_group_check=True)
                        nc.tensor.matmul(pt["Y" + dir_][:], mm(lY), mm(rhs),
                                         start=True, stop=True,
                                         skip_group_check=True)
                    # ---- per-step recurrence ----
                    for j in range(G4):
                        s = s0 + j
                        for dir_ in ("f", "r"):
                            di = 0 if dir_ == "f" else 1
                            lo, hi = (0, 64) if dir_ == "f" else (64, 128)
                            t_proc = s if dir_ == "f" else s_steps - 1 - s
                            mcol = j if dir_ == "f" else G4 - 1 - j
                            blk = slice(mcol * B, (mcol + 1) * B)
                            pX, pY = pt["X" + dir_], pt["Y" + dir_]

                            if s > 0:
                                if layer == 0:
                                    tp = t_proc - 1 if dir_ == "f" else t_proc + 1
                                    hprev = h0_buf[lo:hi, tp * B:(tp + 1) * B]
                                else:
                                    hprev = h1state[lo:hi, :]
                                nc.tensor.matmul(
                                    pX[:, blk], mm(wsb[f"hh{layer}X"][lo:hi, :]),
                                    mm(hprev), start=False, stop=True,
                                    skip_group_check=True)
                                nc.tensor.matmul(
                                    pY[:, blk], mm(wsb[f"hh{layer}Y"][lo:hi, :]),
                                    mm(hprev), start=False, stop=True,
                                    skip_group_check=True)

                            bi = layer * 4 + di * 2
                            bX = bias_sb[:, bi:bi + 1]
                            bY = bias_sb[:, bi + 1:bi + 2]
                            sfi = gat.tile([128, B], EWD, tag=f"sfi{dir_}",
                                           name=f"sfi{dir_}")
                            tog = gat.tile([128, B], EWD, tag=f"tog{dir_}",
                                           name=f"tog{dir_}")
                            nc.scalar.activation(sfi[:], pX[:, blk],
                                                 AF.Sigmoid, bias=bX)
                            nc.scalar.activation(tog[:], pY[:, blk],
                                                 AF.Tanh, bias=bY)

                            Cd = cst[dir_]
                            if s > 0:
                                t1 = tmp.tile([64, B], CD, tag=f"t1{dir_}",
                                              name=f"t1{dir_}")
                                t2 = tmp.tile([64, B], EWD, tag=f"t2{dir_}",
                                              name=f"t2{dir_}")
                                nc.vector.tensor_mul(t1[:], sfi[0:64, :], Cd[:])
                                nc.gpsimd.tensor_mul(t2[:], sfi[64:128, :],
                                                     tog[64:128, :])
                                nc.vector.tensor_add(Cd[:], t1[:], t2[:])
                            else:
                                nc.vector.tensor_mul(Cd[:], sfi[64:128, :],
                                                     tog[64:128, :])
                            tcv = tmp.tile([64, B], EWD, tag=f"tc{dir_}",
                                           name=f"tc{dir_}")
                            nc.scalar.activation(tcv[:], Cd[:], AF.Tanh)

                            # h_stored = (tog_o + 1) * tanh(c)
                            if layer == 0:
                                dst = h0_buf[lo:hi, t_proc * B:(t_proc + 1) * B]
                            elif dir_ == "f" and s == s_steps - 1:
                                dst = h1last[0:64, :]
                            else:
                                dst = h1state[lo:hi, :]
                            nc.vector.scalar_tensor_tensor(
                                dst, tog[0:64, :], 1.0, tcv[:],
                                ALU.add, ALU.mult)
                            if layer == 1 and dir_ == "r" and s == 0:
                                nc.scalar.copy(h1last[64:128, :],
                                               h1state[64:128, :])

            # ---- fc head ----
            pfc = ps.tile([128, G4 * B], f32, tag="pXf")
            nc.tensor.matmul(pfc[0:C_OUT, 0:B], fcT_sb[:], h1last[:],
                             start=True, stop=True, skip_group_check=True)
            osb = tmp.tile([C_OUT, B], f32, tag="osb")
            nc.scalar.activation(osb[:], pfc[0:C_OUT, 0:B], AF.Identity,
                                 bias=fcb_sb[:, 0:1])
            nc.sync.dma_start(out=out_d[:], in_=osb[:])

    _split_multi_waits(nc, mybir)
    return nc


_cached = {}

KERNEL_V2 = True
MM_BF16 = True
EW_BF16 = True
C_BF16 = False


def _get_compiled():
    """Build the Bass module once and compile it into a reusable jitted
    executable over the 8-core mesh (run_bass_via_pjrt re-traces and
    re-lowers on every call; this path does it once per process)."""
    if "exe" in _cached:
        return _cached["exe"]

    import jax
    from jax.sharding import Mesh, NamedSharding, PartitionSpec
    from jax.experimental.shard_map import shard_map as _shard_map
    from concourse import mybir
    from concourse.bass2jax import (_bass_exec_p, install_neuronx_cc_hook,
                                    partition_id_tensor)

    if KERNEL_V2:
        nc = build_nc2(S, mm_bf16=MM_BF16, ew_bf16=EW_BF16, c_bf16=C_BF16)
    else:
        nc = build_nc(S, use_f32r=False)
    install_neuronx_cc_hook()

    partition_name = (nc.partition_id_tensor.name
                      if nc.partition_id_tensor else None)
    in_names, out_names, out_avals, zero_outs = [], [], [], []
    for alloc in nc.m.functions[0].allocations:
        if not isinstance(alloc, mybir.MemoryLocationSet):
            continue
        name = alloc.memorylocations[0].name
        if alloc.kind == "ExternalInput":
            if name != partition_name:
                in_names.append(name)
        elif alloc.kind == "ExternalOutput":
            shape = tuple(alloc.tensor_shape)
            dtype = mybir.dt.np(alloc.dtype)
            out_names.append(name)
            out_avals.append(jax.core.ShapedArray(shape, dtype))
            zero_outs.append(np.zeros(shape, dtype))
    n_params = len(in_names)
    in_names_all = list(in_names) + out_names
    if partition_name is not None:
        in_names_all.append(partition_name)

    def _body(*args):
        operands = list(args)
        if partition_name is not None:
            operands.append(partition_id_tensor())
        return tuple(_bass_exec_p.bind(
            *operands,
            out_avals=tuple(out_avals),
            in_names=tuple(in_names_all),
            out_names=tuple(out_names),
            lowering_input_output_aliases=(),
            sim_require_finite=True,
            sim_require_nnan=True,
            nc=nc,
        ))

    devices = jax.devices()[:NCORES]
    mesh = Mesh(np.asarray(devices), ("core",))
    spec = PartitionSpec("core")
    sharded = jax.jit(
        _shard_map(_body, mesh=mesh,
                   in_specs=(spec,) * (n_params + len(out_avals)),
                   out_specs=(spec,) * len(out_names),
                   check_rep=False),
        keep_unused=True,
    )
    exe = {
        "nc": nc,
        "fn": sharded,
        "in_names": in_names,
        "out_names": out_names,
        "zero_outs": zero_outs,
        "sharding": NamedSharding(mesh, spec),
    }
    _cached["exe"] = exe
    _cached["nc"] = nc
    return exe


def make_in_maps(inputs):
    """Full inputs -> per-core DRAM tensor maps (host-side prep)."""
    if KERNEL_V2:
        shared = _host_prep2(inputs, mm_bf16=MM_BF16)
    else:
        shared = _host_prep(inputs)
    x = np.asarray(inputs["x"], np.float32)
    in_maps = []
    for c in range(NCORES):
        m = dict(shared)
        m["xT"] = _host_xT(x[c * B:(c + 1) * B])
        in_maps.append(m)
    return in_maps


def concat_inputs(exe, in_maps):
    """Per-core maps -> the flat concatenated operand list fed to exe['fn']."""
    per_core = [[np.asarray(m[n]) for n in exe["in_names"]] for m in in_maps]
    ops = [np.concatenate([per_core[c][i] for c in range(NCORES)], axis=0)
           for i in range(len(exe["in_names"]))]
    ops += [np.concatenate([z] * NCORES, axis=0) for z in exe["zero_outs"]]
    return ops


def run_ops(exe, ops):
    """Execute; returns the full [FULL_B, C_OUT] output."""
    import jax
    outs = exe["fn"](*ops)
    jax.block_until_ready(outs)
    # out tensor is [C_OUT, B] per core, concatenated on axis 0 over cores.
    o = np.asarray(outs[exe["out_names"].index("out")])
    o = o.reshape(NCORES, C_OUT, B)
    return np.ascontiguousarray(
        np.concatenate([o[c].T for c in range(NCORES)], axis=0)
        .astype(np.float32))


def kernel(**inputs):
    exe = _get_compiled()
    return run_ops(exe, concat_inputs(exe, make_in_maps(inputs)))

